# revision 1
# baseline (speedup 1.0000x reference)
"""Trainium2 Bass kernel for nn_MultiHeadAttention (B=2, S=4096, D=1024, H=16, Dh=64).

Sharding over 8 cores: core c handles batch b=c//4 and head-group hg=c%4
(4 heads = 256 channels). Host gathers by summing the 4 per-head-group partial
output projections per batch (row-parallel output projection).

Per-core device pipeline (all matmuls bf16, fp32 PSUM accumulation):
  phase A: Q/K/V projections.
       QT/KT produced as [256ch -> 2 "pair" tensors of 128 part, 4096 tok]
       V produced as [128 tok-part, 32 chunk, 256 ch]  (natural AV layout)
  phase B: attention per (head-pair, q-block 1024):
       QK^T:  lhsT=KT[64d,128k] rhs=QT[64d,1024q] -> ST [128k, 1024q] psum,
              two heads run concurrently on disjoint PE row-groups.
       exp:   ACT activation Exp (scale=1/8) PSUM->SBUF bf16  (ET [k,q])
       AV:    lhsT=V[128k,64d] rhs=ET[128k,512q] -> attnT [128d2, q] psum,
              two heads concurrent on disjoint PE col-groups.
       rowsum: DVE bf16 halving tree over k-chunks + PE ones-matmul 128->1.
       normalize: attnT = attn_psum / rowsum_bcast (PE K=1 broadcast + DVE divide)
  phase C: output projection out[t,o] = sum_c attnT[c,t] WoR[c,o] + bo.
"""

import math
import os
import sys
import functools

import numpy as np
import ml_dtypes

sys.path.insert(0, "/opt/trn_rl_repo")

import concourse.bass as bass  # noqa: E402
import concourse.mybir as mybir  # noqa: E402
import concourse.tile as tile  # noqa: E402
from concourse import bass_utils  # noqa: E402

B, S, D, H, DH = 2, 4096, 1024, 16, 64
NCORES = 8
HG = 4  # head groups (cores per batch)
OC = 256  # q/k/v channels per core
BF16 = mybir.dt.bfloat16
F32 = mybir.dt.float32
QBLK = 1024
NQB = S // QBLK  # 4
NKT = S // 128  # 32 k-tiles
NTT = S // 128  # 32 t-tiles
bf16 = ml_dtypes.bfloat16


_TPB_ENGINES = None


def _split_waits(nc, max_waits=1):
    """walrus codegen in this container rejects TPB instructions carrying more
    than one sync-wait command.  Spill extra semaphore waits onto preceding
    NoOps on the same engine (engines execute their queue in order, so a NoOp
    that waits immediately before the instruction is equivalent)."""
    import bass_rust

    global _TPB_ENGINES
    if _TPB_ENGINES is None:
        _TPB_ENGINES = {
            mybir.EngineType.Pool,
            mybir.EngineType.Activation,
            mybir.EngineType.PE,
            mybir.EngineType.DVE,
            mybir.EngineType.SP,
        }
    ctr = 0
    for bb in nc.main_func.blocks:
        insts = bb.instructions
        out = []
        changed = False
        for inst in insts:
            si = getattr(inst, "sync_info", None)
            if (
                si is not None
                and si.on_wait
                and len(si.on_wait) > max_waits
                and inst.engine in _TPB_ENGINES
            ):
                waits = list(si.on_wait)
                keep = waits[-max_waits:]
                spill = waits[:-max_waits]
                for i in range(0, len(spill), max_waits):
                    nop = bass_rust.InstNoOp(
                        name=f"{inst.name}-sw{ctr}", ins=[], outs=[]
                    )
                    ctr += 1
                    nop.engine = inst.engine
                    nop.sync_info = mybir.SyncInfo(
                        on_wait=spill[i : i + max_waits], on_update=[]
                    )
                    out.append(nop)
                inst.sync_info = mybir.SyncInfo(
                    on_wait=keep, on_update=list(si.on_update)
                )
                changed = True
            out.append(inst)
        if changed:
            insts[:] = out
    return nc


@functools.lru_cache(maxsize=4)
def _build(masked: bool, split_waits: bool = True):
    nc = bass.Bass()

    xqT_d = nc.dram_tensor("xqT", [D, S], BF16, kind="ExternalInput")
    xkT_d = nc.dram_tensor("xkT", [D, S], BF16, kind="ExternalInput")
    xvT_d = nc.dram_tensor("xvT", [D, S], BF16, kind="ExternalInput")
    wqT_d = nc.dram_tensor("wqT", [D, OC], BF16, kind="ExternalInput")
    wkT_d = nc.dram_tensor("wkT", [D, OC], BF16, kind="ExternalInput")
    wvT_d = nc.dram_tensor("wvT", [D, OC], BF16, kind="ExternalInput")
    bq_d = nc.dram_tensor("bq2", [128, 2], F32, kind="ExternalInput")
    bk_d = nc.dram_tensor("bk2", [128, 2], F32, kind="ExternalInput")
    bv_d = nc.dram_tensor("bvr", [1, OC], F32, kind="ExternalInput")
    woR_d = nc.dram_tensor("woR", [OC, D], BF16, kind="ExternalInput")
    bo_d = nc.dram_tensor("bor", [1, D], F32, kind="ExternalInput")
    if masked:
        maskT_d = nc.dram_tensor("maskT", [S, S], BF16, kind="ExternalInput")
    out_d = nc.dram_tensor("out", [S, D], F32, kind="ExternalOutput")

    with tile.TileContext(nc) as tc:
        with (
            tc.tile_pool(name="persist", bufs=1) as persist,
            tc.tile_pool(name="wpool", bufs=1) as wpool,
            tc.tile_pool(name="xt", bufs=2) as xtp,
            tc.tile_pool(name="et", bufs=6) as etp,
            tc.tile_pool(name="rtmp", bufs=3) as rtmp,
            tc.tile_pool(name="rrow", bufs=2) as rrow,
            tc.tile_pool(name="outp", bufs=2) as outp,
            tc.tile_pool(name="stps", bufs=2, space="PSUM") as stps,
            tc.tile_pool(name="stps2", bufs=1, space="PSUM") as stps2,
            tc.tile_pool(name="avps", bufs=1, space="PSUM") as avps,
        ):
            # persistent SBUF tensors
            QT = [persist.tile([128, S], BF16, tag=f"QT{p}", name=f"QT{p}") for p in range(2)]
            KT = [persist.tile([128, S], BF16, tag=f"KT{p}", name=f"KT{p}") for p in range(2)]
            Vsb = persist.tile([128, NKT, OC], BF16, tag="Vsb")
            attnT = [persist.tile([128, S], BF16, tag=f"attnT{p}", name=f"attnT{p}") for p in range(2)]
            ones_row = persist.tile([1, 128], F32, tag="ones_row")
            ones64 = persist.tile([128, 64], BF16, tag="ones64")
            zeros128 = persist.tile([128, 128], BF16, tag="zeros128")
            bv_bc = persist.tile([128, OC], F32, tag="bv_bc")
            bo_bc = persist.tile([128, D], F32, tag="bo_bc")
            nc.gpsimd.memset(ones_row[:], 1.0)
            nc.gpsimd.memset(ones64[:], 1.0)
            nc.gpsimd.memset(zeros128[:], 0.0)

            wq_sb = wpool.tile([128, 8, OC], BF16, tag="wq")
            wk_sb = wpool.tile([128, 8, OC], BF16, tag="wk")
            wv_sb = wpool.tile([128, 8, OC], BF16, tag="wv")
            wo_sb = wpool.tile([128, 2, D], BF16, tag="wo")
            bq_sb = wpool.tile([128, 2], F32, tag="bq")
            bk_sb = wpool.tile([128, 2], F32, tag="bk")
            bv_sb = wpool.tile([1, OC], F32, tag="bv")
            bo_sb = wpool.tile([1, D], F32, tag="bo")
            nc.sync.dma_start(wq_sb[:], wqT_d.rearrange("(dc p) o -> p dc o", p=128))
            nc.sync.dma_start(wk_sb[:], wkT_d.rearrange("(dc p) o -> p dc o", p=128))
            nc.sync.dma_start(wv_sb[:], wvT_d.rearrange("(dc p) o -> p dc o", p=128))
            nc.sync.dma_start(wo_sb[:], woR_d.rearrange("(cc p) o -> p cc o", p=128))
            nc.sync.dma_start(bq_sb[:], bq_d[:])
            nc.sync.dma_start(bk_sb[:], bk_d[:])
            nc.sync.dma_start(bv_sb[:], bv_d[:])
            nc.sync.dma_start(bo_sb[:], bo_d[:])

            # broadcast bv / bo across partitions via K=1 matmuls
            ps = stps.tile([128, QBLK], F32, tag="st", name="bvbc")
            nc.tensor.matmul(ps[:, 0:OC], ones_row[:], bv_sb[:], start=True, stop=True)
            nc.vector.tensor_copy(bv_bc[:], ps[:, 0:OC])
            ps = stps.tile([128, QBLK], F32, tag="st", name="bobc")
            for oh in range(2):
                nc.tensor.matmul(
                    ps[:, oh * 512 : (oh + 1) * 512],
                    ones_row[:],
                    bo_sb[:, oh * 512 : (oh + 1) * 512],
                    start=True,
                    stop=True,
                )
            nc.vector.tensor_copy(bo_bc[:], ps[:])

            # ---------------- projections ----------------
            for which, (xd, wsb, bsb, dst) in enumerate(
                (
                    (xqT_d, wq_sb, bq_sb, QT),
                    (xkT_d, wk_sb, bk_sb, KT),
                )
            ):
                xr = xd.rearrange("(dc p) t -> p dc t", p=128)
                for tt in range(8):  # 512-token tiles
                    xt = xtp.tile([128, 8, 512], BF16, tag="xqk", name="xqk")
                    nc.sync.dma_start(xt[:], xr[:, :, tt * 512 : (tt + 1) * 512])
                    for oc in range(2):
                        pst = stps.tile([128, QBLK], F32, tag="st", name="pqk")
                        for dc in range(8):
                            nc.tensor.matmul(
                                pst[:, 0:512],
                                wsb[:, dc, oc * 128 : (oc + 1) * 128],
                                xt[:, dc, :],
                                start=(dc == 0),
                                stop=(dc == 7),
                            )
                        nc.vector.tensor_scalar_add(
                            dst[oc][:, tt * 512 : (tt + 1) * 512],
                            pst[:, 0:512],
                            bsb[:, oc : oc + 1],
                        )

            # V projection -> Vsb [128 tok, chunk, 256]
            xvr = xvT_d.rearrange("(dc p) t -> p dc t", p=128)
            for tcI in range(NKT):
                xt = xtp.tile([128, 8, 128], BF16, tag="xv", name="xv")
                nc.sync.dma_start(xt[:], xvr[:, :, tcI * 128 : (tcI + 1) * 128])
                pst = stps2.tile([128, QBLK], F32, tag="st", name="pv")
                for dc in range(8):
                    nc.tensor.matmul(
                        pst[:, 0:OC],
                        xt[:, dc, :],
                        wv_sb[:, dc, :],
                        start=(dc == 0),
                        stop=(dc == 7),
                    )
                nc.vector.tensor_add(Vsb[:, tcI, :], pst[:, 0:OC], bv_bc[:])

            # ---------------- attention + output projection ----------------
            if masked:
                mrr = maskT_d.rearrange("(kt p) q -> kt p q", p=128)
            our = out_d.rearrange("(tt p) o -> tt p o", p=128)
            for qb in range(NQB):
                q0 = qb * QBLK
                for pair in range(2):
                    QTp, KTp, ATp = QT[pair], KT[pair], attnT[pair]
                    attn_ps = avps.tile([128, QBLK], F32, tag="attn", name="attn")
                    # zero-fill both banks ([128,128]-mode zero matmuls) so the
                    # two heads can accumulate with start=False in any order.
                    for qh in range(2):
                        nc.tensor.matmul(
                            attn_ps[:, qh * 512 : (qh + 1) * 512],
                            zeros128[:],
                            QTp[:, q0 + qh * 512 : q0 + (qh + 1) * 512],
                            start=True,
                            stop=False,
                        )
                    rs5 = [None, None]
                    et_cur = [None, None]
                    for kt in range(NKT):
                        ki = kt % 4
                        sts = [None, None]
                        for h2 in range(2):
                            if ki == 0:
                                et_cur[h2] = etp.tile([128, 4, QBLK], BF16, tag="et", name="et")
                            slot = (kt * 2 + h2) % 3
                            sp = stps2 if slot == 2 else stps
                            sts[h2] = sp.tile([128, QBLK], F32, tag="st", name="st")
                        for qh in range(2):
                            for h2 in range(2):
                                b0 = h2 * 64
                                nc.tensor.matmul(
                                    sts[h2][:, qh * 512 : (qh + 1) * 512],
                                    KTp[b0 : b0 + 64, kt * 128 : (kt + 1) * 128],
                                    QTp[b0 : b0 + 64, q0 + qh * 512 : q0 + (qh + 1) * 512],
                                    start=True,
                                    stop=True,
                                )
                        for h2 in range(2):
                            et = et_cur[h2]
                            nc.scalar.activation(
                                et[:, ki, :],
                                sts[h2][:],
                                mybir.ActivationFunctionType.Exp,
                                scale=1.0 / math.sqrt(DH),
                            )
                            if masked:
                                mk = rtmp.tile([128, QBLK], BF16, tag="mk", name="mk")
                                nc.sync.dma_start(mk[:], mrr[kt][:, q0 : q0 + QBLK])
                                nc.vector.tensor_mul(et[:, ki, :], et[:, ki, :], mk[:])
                        # AV accumulate (deprioritized: fills PE gaps)
                        with tc.high_priority(offset=-8000):
                            for qh in range(2):
                                for h2 in range(2):
                                    b0 = h2 * 64
                                    h_local = pair * 2 + h2
                                    nc.tensor.matmul(
                                        attn_ps[b0 : b0 + 64, qh * 512 : (qh + 1) * 512],
                                        Vsb[:, kt, h_local * 64 : (h_local + 1) * 64],
                                        et_cur[h2][:, ki, qh * 512 : (qh + 1) * 512],
                                        start=False,
                                        stop=False,
                                    )
                        # rowsum tree contribution once per 4-k-tile group
                        if ki == 3:
                            for h2 in range(2):
                                et = et_cur[h2]
                                t2 = rtmp.tile([128, 2, QBLK], BF16, tag="t2", name="t2")
                                nc.vector.tensor_add(t2[:], et[:, 0:2, :], et[:, 2:4, :])
                                if kt == 3:
                                    rs5[h2] = rtmp.tile([128, QBLK], BF16, tag="rs5", name="rs5")
                                    nc.vector.tensor_add(rs5[h2][:], t2[:, 0, :], t2[:, 1, :])
                                else:
                                    ts_ = rtmp.tile([128, QBLK], BF16, tag="ts", name="ts")
                                    nc.vector.tensor_add(ts_[:], t2[:, 0, :], t2[:, 1, :])
                                    nc.vector.tensor_add(rs5[h2][:], rs5[h2][:], ts_[:])
                    with tc.high_priority(offset=-8000):
                        # close both banks' accumulation groups
                        for qh in range(2):
                            nc.tensor.matmul(
                                attn_ps[:, qh * 512 : (qh + 1) * 512],
                                zeros128[:],
                                QTp[:, q0 + qh * 512 : q0 + (qh + 1) * 512],
                                start=False,
                                stop=True,
                            )
                        # rowsums + free broadcast via ones[128,64] matmuls
                        st_rs = stps.tile([128, QBLK], F32, tag="st", name="st_rs")
                        for qh in range(2):
                            for h2 in range(2):
                                qx = (qh + h2) % 2
                                nc.tensor.matmul(
                                    st_rs[h2 * 64 : h2 * 64 + 64, qx * 512 : qx * 512 + 512],
                                    ones64[:],
                                    rs5[h2][:, qx * 512 : qx * 512 + 512],
                                    start=True,
                                    stop=True,
                                )
                        rs_bc = rrow.tile([128, QBLK], F32, tag="rsbc", name="rsbc")
                        nc.vector.tensor_copy(rs_bc[:], st_rs[:])
                        nc.vector.reciprocal(rs_bc[:], rs_bc[:])
                        nc.vector.tensor_tensor(
                            ATp[:, q0 : q0 + QBLK],
                            attn_ps[:],
                            rs_bc[:],
                            mybir.AluOpType.mult,
                        )
                # output projection for this q-block's token tiles
                with tc.high_priority(offset=-8000):
                    for tt in range(qb * 8, (qb + 1) * 8):
                        ps = stps2.tile([128, QBLK], F32, tag="st", name="po")
                        for oh in range(2):
                            for cc in range(2):
                                nc.tensor.matmul(
                                    ps[:, oh * 512 : (oh + 1) * 512],
                                    attnT[cc][:, tt * 128 : (tt + 1) * 128],
                                    wo_sb[:, cc, oh * 512 : (oh + 1) * 512],
                                    start=(cc == 0),
                                    stop=(cc == 1),
                                )
                        ot = outp.tile([128, D], F32, tag="ot", name="ot")
                        nc.vector.tensor_add(ot[:], ps[:], bo_bc[:])
                        nc.sync.dma_start(our[tt], ot[:])

    return _split_waits(nc) if split_waits else nc


def _prep_in_maps(inputs):
    q = np.asarray(inputs["query"], np.float32)
    k = np.asarray(inputs["key"], np.float32)
    v = np.asarray(inputs["value"], np.float32)
    mask = np.asarray(inputs["mask"])
    Wq = np.asarray(inputs["Wq"], np.float32)
    Wk = np.asarray(inputs["Wk"], np.float32)
    Wv = np.asarray(inputs["Wv"], np.float32)
    Wo = np.asarray(inputs["Wo"], np.float32)
    bq = np.asarray(inputs["bq"], np.float32)
    bk = np.asarray(inputs["bk"], np.float32)
    bv = np.asarray(inputs["bv"], np.float32)
    bo = np.asarray(inputs["bo"], np.float32)

    masked = not bool((mask != 0).all())
    xT = {}
    for nm, x in (("q", q), ("k", k), ("v", v)):
        for b in range(B):
            xT[(nm, b)] = np.ascontiguousarray(x[b].T).astype(bf16)
    if masked:
        maskT = np.ascontiguousarray(
            (np.broadcast_to(mask[0, 0], (S, S)).T != 0)
        ).astype(bf16)

    in_maps = []
    for c in range(NCORES):
        b, hg = c // HG, c % HG
        sl = slice(hg * OC, (hg + 1) * OC)
        m = {
            "xqT": xT[("q", b)],
            "xkT": xT[("k", b)],
            "xvT": xT[("v", b)],
            "wqT": np.ascontiguousarray(Wq[sl].T).astype(bf16),
            "wkT": np.ascontiguousarray(Wk[sl].T).astype(bf16),
            "wvT": np.ascontiguousarray(Wv[sl].T).astype(bf16),
            "bq2": np.ascontiguousarray(bq[sl].reshape(2, 128).T),
            "bk2": np.ascontiguousarray(bk[sl].reshape(2, 128).T),
            "bvr": bv[sl].reshape(1, OC).copy(),
            "woR": np.ascontiguousarray(Wo[:, sl].T).astype(bf16),
            "bor": (bo if hg == 0 else np.zeros_like(bo)).reshape(1, D).copy(),
        }
        if masked:
            m["maskT"] = maskT
        in_maps.append(m)
    return in_maps, masked


def _install_profile_hook():
    """Provide antenv.axon_hooks + register the NTFF profile hook via ctypes
    against libaxon_pjrt.so (the agent image lacks antenv.axon_hooks, which
    makes run_bass_kernel_spmd(trace=True) fall over; see trn_boot.py)."""
    import types
    import ctypes
    import contextlib

    if "antenv.axon_hooks" in sys.modules:
        return
    mod = types.ModuleType("antenv.axon_hooks")
    state = {"hook": None}
    mod.set_axon_ntff_profile_hook = lambda h: state.__setitem__("hook", h)
    mod.get_axon_ntff_profile_hook = lambda: state["hook"]
    sys.modules["antenv.axon_hooks"] = mod

    so_path = "/opt/axon/libaxon_pjrt.so"
    if not os.path.exists(so_path):
        return
    lib = ctypes.CDLL(so_path)
    if not hasattr(lib, "axon_start_nrt_profile"):
        return
    lib.axon_start_nrt_profile.argtypes = [
        ctypes.POINTER(ctypes.c_int64),
        ctypes.c_size_t,
    ]
    lib.axon_start_nrt_profile.restype = ctypes.c_int64
    lib.axon_stop_nrt_profile.argtypes = [ctypes.c_char_p]
    lib.axon_stop_nrt_profile.restype = ctypes.c_int64

    @contextlib.contextmanager
    def _hook(output_dir, device_ids):
        import jax

        jax.devices()
        if device_ids:
            ids = (ctypes.c_int64 * len(device_ids))(*device_ids)
            rc = lib.axon_start_nrt_profile(ids, len(device_ids))
        else:
            rc = lib.axon_start_nrt_profile(None, 0)
        if rc != 0:
            raise RuntimeError(f"axon_start_nrt_profile rc={rc}")
        try:
            yield
        finally:
            n = lib.axon_stop_nrt_profile(str(output_dir).encode())
            print(f"profile: {n} file(s) written to {output_dir}", file=sys.stderr)

    mod.set_axon_ntff_profile_hook(_hook)


def run(inputs, trace=False):
    if trace:
        _install_profile_hook()
    in_maps, masked = _prep_in_maps(inputs)
    nc = _build(masked)
    res = bass_utils.run_bass_kernel_spmd(
        nc, in_maps, core_ids=list(range(NCORES)), trace=trace
    )
    out = np.zeros((B, S, D), np.float32)
    for c in range(NCORES):
        out[c // HG] += res.results[c]["out"]
    return out, res


def kernel(**inputs):
    return run(inputs, trace=False)[0]



# revision 2
# speedup vs baseline: 1.4894x; 1.4894x over previous
"""Trainium2 Bass kernel for nn_MultiHeadAttention (B=2, S=4096, D=1024, H=16, Dh=64).

Sharding over 8 cores: core c handles batch b=c//4 and head-group hg=c%4
(4 heads = 256 channels). Host gathers by summing the 4 per-head-group partial
output projections per batch (row-parallel output projection).

Per-core device pipeline (all matmuls bf16, fp32 PSUM accumulation), built
around keeping the ACT (scalar) engine 100% busy on the softmax exp — the
hard floor for this problem (~560us of exp at 1 elem/cycle/lane).

q-blocks of 512 tokens; per (qb, pair-of-heads) block, per k-tile kt (128 keys):
  QK^T: 2 matmuls (one per head), lhsT=KT[64d,128k] rhs=QT[64d,512q]
        -> ST [128k, 2*512q] psum; the two heads use disjoint PE row groups
        (K=64 at row offset 0/64) and run concurrently (~126ns/MM measured).
  exp:  one ACT instruction over the whole [128,1024] ST tile -> ET bf16.
  AV:   2 matmuls (one per head), lhsT=V[128k,64d] rhs=ET[128k,512q]
        accumulated into attn psum [2*64d, 512q]; disjoint PE col groups,
        concurrent. start/stop via kt==0/31 (no zero-fill pass).
  rowsum: DVE bf16 halving tree over the 32 ET k-planes.
Block end: ones-matmul reduces+broadcasts rowsums, DVE reciprocal,
normalize attn -> attnT (bf16). Out-projection per 512-token q-block:
out[t,o] = sum_c attnT[c,t] WoR[c,o] + bo, streamed to HBM.
"""

import math
import os
import sys
import functools

import numpy as np
import ml_dtypes

sys.path.insert(0, "/opt/trn_rl_repo")

import concourse.bass as bass  # noqa: E402
import concourse.mybir as mybir  # noqa: E402
import concourse.tile as tile  # noqa: E402
from concourse import bass_utils  # noqa: E402

B, S, D, H, DH = 2, 4096, 1024, 16, 64
NCORES = 8
HG = 4  # head groups (cores per batch)
OC = 256  # q/k/v channels per core
BF16 = mybir.dt.bfloat16
F32 = mybir.dt.float32
QBLK = 512
NQB = S // QBLK  # 8
NKT = S // 128  # 32 k-tiles
bf16 = ml_dtypes.bfloat16


_TPB_ENGINES = None


def _split_waits(nc, max_waits=1):
    """walrus codegen in this container rejects TPB instructions carrying more
    than one sync-wait command.  Spill extra semaphore waits onto preceding
    NoOps on the same engine (engines execute their queue in order, so a NoOp
    that waits immediately before the instruction is equivalent)."""
    import bass_rust

    global _TPB_ENGINES
    if _TPB_ENGINES is None:
        _TPB_ENGINES = {
            mybir.EngineType.Pool,
            mybir.EngineType.Activation,
            mybir.EngineType.PE,
            mybir.EngineType.DVE,
            mybir.EngineType.SP,
        }
    ctr = 0
    for bb in nc.main_func.blocks:
        insts = bb.instructions
        out = []
        changed = False
        for inst in insts:
            si = getattr(inst, "sync_info", None)
            if (
                si is not None
                and si.on_wait
                and len(si.on_wait) > max_waits
                and inst.engine in _TPB_ENGINES
            ):
                waits = list(si.on_wait)
                keep = waits[-max_waits:]
                spill = waits[:-max_waits]
                for i in range(0, len(spill), max_waits):
                    nop = bass_rust.InstNoOp(
                        name=f"{inst.name}-sw{ctr}", ins=[], outs=[]
                    )
                    ctr += 1
                    nop.engine = inst.engine
                    nop.sync_info = mybir.SyncInfo(
                        on_wait=spill[i : i + max_waits], on_update=[]
                    )
                    out.append(nop)
                inst.sync_info = mybir.SyncInfo(
                    on_wait=keep, on_update=list(si.on_update)
                )
                changed = True
            out.append(inst)
        if changed:
            insts[:] = out
    return nc


@functools.lru_cache(maxsize=4)
def _build(masked: bool, split_waits: bool = True):
    nc = bass.Bass()

    xqT_d = nc.dram_tensor("xqT", [D, S], BF16, kind="ExternalInput")
    xkT_d = nc.dram_tensor("xkT", [D, S], BF16, kind="ExternalInput")
    xvT_d = nc.dram_tensor("xvT", [D, S], BF16, kind="ExternalInput")
    wqT_d = nc.dram_tensor("wqT", [D, OC], BF16, kind="ExternalInput")
    wkT_d = nc.dram_tensor("wkT", [D, OC], BF16, kind="ExternalInput")
    wvT_d = nc.dram_tensor("wvT", [D, OC], BF16, kind="ExternalInput")
    bq_d = nc.dram_tensor("bq2", [128, 2], F32, kind="ExternalInput")
    bk_d = nc.dram_tensor("bk2", [128, 2], F32, kind="ExternalInput")
    bv_d = nc.dram_tensor("bvr", [1, OC], F32, kind="ExternalInput")
    woR_d = nc.dram_tensor("woR", [OC, D], BF16, kind="ExternalInput")
    bo_d = nc.dram_tensor("bor", [1, D], F32, kind="ExternalInput")
    if masked:
        maskT_d = nc.dram_tensor("maskT", [S, S], BF16, kind="ExternalInput")
    out_d = nc.dram_tensor("out", [S, D], F32, kind="ExternalOutput")

    with tile.TileContext(nc) as tc:
        with (
            tc.tile_pool(name="persist", bufs=1) as persist,
            tc.tile_pool(name="wpool", bufs=1) as wpool,
            tc.tile_pool(name="xt", bufs=2) as xtp,
            tc.tile_pool(name="et", bufs=5) as etp,
            tc.tile_pool(name="rtmp", bufs=3) as rtmp,
            tc.tile_pool(name="rrow", bufs=2) as rrow,
            tc.tile_pool(name="outp", bufs=2) as outp,
            tc.tile_pool(name="stp", bufs=2, space="PSUM") as stp,
            tc.tile_pool(name="avp", bufs=1, space="PSUM") as avp,
            tc.tile_pool(name="misc", bufs=3, space="PSUM") as miscp,
        ):
            # persistent SBUF tensors
            QT = [persist.tile([128, S], BF16, tag=f"QT{p}", name=f"QT{p}") for p in range(2)]
            KT = [persist.tile([128, S], BF16, tag=f"KT{p}", name=f"KT{p}") for p in range(2)]
            Vsb = persist.tile([128, NKT, OC], BF16, tag="Vsb")
            attnT = [persist.tile([128, S], BF16, tag=f"attnT{p}", name=f"attnT{p}") for p in range(2)]
            ones_row = persist.tile([1, 128], F32, tag="ones_row")
            ones64 = persist.tile([128, 64], BF16, tag="ones64")
            bv_bc = persist.tile([128, OC], F32, tag="bv_bc")
            bo_bc = persist.tile([128, D], F32, tag="bo_bc")
            nc.gpsimd.memset(ones_row[:], 1.0)
            nc.gpsimd.memset(ones64[:], 1.0)

            wq_sb = wpool.tile([128, 8, OC], BF16, tag="wq")
            wk_sb = wpool.tile([128, 8, OC], BF16, tag="wk")
            wv_sb = wpool.tile([128, 8, OC], BF16, tag="wv")
            wo_sb = wpool.tile([128, 2, D], BF16, tag="wo")
            bq_sb = wpool.tile([128, 2], F32, tag="bq")
            bk_sb = wpool.tile([128, 2], F32, tag="bk")
            bv_sb = wpool.tile([1, OC], F32, tag="bv")
            bo_sb = wpool.tile([1, D], F32, tag="bo")
            nc.sync.dma_start(wq_sb[:], wqT_d.rearrange("(dc p) o -> p dc o", p=128))
            nc.sync.dma_start(wk_sb[:], wkT_d.rearrange("(dc p) o -> p dc o", p=128))
            nc.sync.dma_start(wv_sb[:], wvT_d.rearrange("(dc p) o -> p dc o", p=128))
            nc.sync.dma_start(wo_sb[:], woR_d.rearrange("(cc p) o -> p cc o", p=128))
            nc.sync.dma_start(bq_sb[:], bq_d[:])
            nc.sync.dma_start(bk_sb[:], bk_d[:])
            nc.sync.dma_start(bv_sb[:], bv_d[:])
            nc.sync.dma_start(bo_sb[:], bo_d[:])

            # warm the ACT exp table while projections run (off critical path)
            warm = rtmp.tile([1, 2], F32, tag="warm", name="warm")
            nc.scalar.activation(
                warm[:], bq_sb[0:1, :], mybir.ActivationFunctionType.Exp
            )

            # broadcast bv / bo across partitions via K=1 matmuls
            ps = miscp.tile([128, 512], F32, tag="ms", name="bvbc")
            nc.tensor.matmul(ps[:, 0:OC], ones_row[:], bv_sb[:], start=True, stop=True)
            nc.vector.tensor_copy(bv_bc[:], ps[:, 0:OC])
            for oh in range(2):
                ps = miscp.tile([128, 512], F32, tag="ms", name="bobc")
                nc.tensor.matmul(
                    ps[:],
                    ones_row[:],
                    bo_sb[:, oh * 512 : (oh + 1) * 512],
                    start=True,
                    stop=True,
                )
                nc.vector.tensor_copy(bo_bc[:, oh * 512 : (oh + 1) * 512], ps[:])

            # ---------------- Q/K projections ----------------
            for which, (xd, wsb, bsb, dst) in enumerate(
                (
                    (xkT_d, wk_sb, bk_sb, KT),
                    (xqT_d, wq_sb, bq_sb, QT),
                )
            ):
                xr = xd.rearrange("(dc p) t -> p dc t", p=128)
                for tt in range(8):  # 512-token tiles
                    xt = xtp.tile([128, 8, 512], BF16, tag="xqk", name="xqk")
                    nc.sync.dma_start(xt[:], xr[:, :, tt * 512 : (tt + 1) * 512])
                    for oc in range(2):
                        pst = miscp.tile([128, 512], F32, tag="ms", name="pqk")
                        for dc in range(8):
                            nc.tensor.matmul(
                                pst[:],
                                wsb[:, dc, oc * 128 : (oc + 1) * 128],
                                xt[:, dc, :],
                                start=(dc == 0),
                                stop=(dc == 7),
                            )
                        nc.vector.tensor_scalar_add(
                            dst[oc][:, tt * 512 : (tt + 1) * 512],
                            pst[:],
                            bsb[:, oc : oc + 1],
                        )

            # ---------------- V projection (fills PE gaps during attention) --
            xvr = xvT_d.rearrange("(dc p) t -> p dc t", p=128)
            with tc.high_priority(offset=-2000):
                for tcI in range(NKT):
                    xt = xtp.tile([128, 8, 128], BF16, tag="xv", name="xv")
                    nc.sync.dma_start(xt[:], xvr[:, :, tcI * 128 : (tcI + 1) * 128])
                    pst = miscp.tile([128, 512], F32, tag="ms", name="pv")
                    for dc in range(8):
                        nc.tensor.matmul(
                            pst[:, 0:OC],
                            xt[:, dc, :],
                            wv_sb[:, dc, :],
                            start=(dc == 0),
                            stop=(dc == 7),
                        )
                    nc.vector.tensor_add(Vsb[:, tcI, :], pst[:, 0:OC], bv_bc[:])

            # ---------------- attention + output projection ----------------
            if masked:
                mrr = maskT_d.rearrange("(kt p) q -> kt p q", p=128)
            our = out_d.rearrange("(tt p) o -> tt p o", p=128)
            for qb in range(NQB):
                q0 = qb * QBLK
                for pair in range(2):
                    QTp, KTp, ATp = QT[pair], KT[pair], attnT[pair]
                    attn_ps = avp.tile([128, QBLK], F32, tag="attn", name="attn")
                    rs5 = None
                    et = None
                    for kt in range(NKT):
                        ki = kt % 4
                        if ki == 0:
                            et = etp.tile([128, 4, 1024], BF16, tag="et", name="et")
                        st = stp.tile([128, 1024], F32, tag="st", name="st")
                        for h2 in range(2):
                            b0 = h2 * 64
                            nc.tensor.matmul(
                                st[:, h2 * 512 : (h2 + 1) * 512],
                                KTp[b0 : b0 + 64, kt * 128 : (kt + 1) * 128],
                                QTp[b0 : b0 + 64, q0 : q0 + QBLK],
                                start=True,
                                stop=True,
                            )
                        nc.scalar.activation(
                            et[:, ki, :],
                            st[:],
                            mybir.ActivationFunctionType.Exp,
                            scale=1.0 / math.sqrt(DH),
                        )
                        if masked:
                            mk = rtmp.tile([128, 512], BF16, tag="mk", name="mk")
                            nc.sync.dma_start(mk[:], mrr[kt][:, q0 : q0 + QBLK])
                            for h2 in range(2):
                                nc.vector.tensor_mul(
                                    et[:, ki, h2 * 512 : (h2 + 1) * 512],
                                    et[:, ki, h2 * 512 : (h2 + 1) * 512],
                                    mk[:],
                                )
                        # AV accumulate (deprioritized: fills PE gaps)
                        with tc.high_priority(offset=-8000):
                            for h2 in range(2):
                                b0 = h2 * 64
                                h_local = pair * 2 + h2
                                nc.tensor.matmul(
                                    attn_ps[b0 : b0 + 64, :],
                                    Vsb[:, kt, h_local * 64 : (h_local + 1) * 64],
                                    et[:, ki, h2 * 512 : (h2 + 1) * 512],
                                    start=(kt == 0),
                                    stop=(kt == NKT - 1),
                                    skip_group_check=True,
                                )
                        # rowsum tree contribution once per 4-k-tile group
                        if ki == 3:
                            t2 = rtmp.tile([128, 2, 1024], BF16, tag="t2", name="t2")
                            nc.vector.tensor_add(t2[:], et[:, 0:2, :], et[:, 2:4, :])
                            if kt == 3:
                                rs5 = rrow.tile([128, 1024], BF16, tag="rs5", name="rs5")
                                nc.vector.tensor_add(rs5[:], t2[:, 0, :], t2[:, 1, :])
                            else:
                                ts_ = rtmp.tile([128, 1024], BF16, tag="ts", name="ts")
                                nc.vector.tensor_add(ts_[:], t2[:, 0, :], t2[:, 1, :])
                                nc.vector.tensor_add(rs5[:], rs5[:], ts_[:])
                    # block end: rowsum reduce+broadcast, reciprocal, normalize
                    with tc.high_priority(offset=-8000):
                        st_rs = miscp.tile([128, 512], F32, tag="ms", name="st_rs")
                        for h2 in range(2):
                            nc.tensor.matmul(
                                st_rs[h2 * 64 : h2 * 64 + 64, :],
                                ones64[:],
                                rs5[:, h2 * 512 : (h2 + 1) * 512],
                                start=True,
                                stop=True,
                            )
                        rs_bc = rrow.tile([128, 512], F32, tag="rsbc", name="rsbc")
                        nc.vector.reciprocal(rs_bc[:], st_rs[:])
                        nc.vector.tensor_tensor(
                            ATp[:, q0 : q0 + QBLK],
                            attn_ps[:],
                            rs_bc[:],
                            mybir.AluOpType.mult,
                        )
                # output projection for this q-block's token tiles
                with tc.high_priority(offset=-8000):
                    for tt in range(qb * 4, (qb + 1) * 4):
                        ot = outp.tile([128, D], F32, tag="ot", name="ot")
                        for oh in range(2):
                            ps = miscp.tile([128, 512], F32, tag="ms", name="po")
                            for cc in range(2):
                                nc.tensor.matmul(
                                    ps[:],
                                    attnT[cc][:, tt * 128 : (tt + 1) * 128],
                                    wo_sb[:, cc, oh * 512 : (oh + 1) * 512],
                                    start=(cc == 0),
                                    stop=(cc == 1),
                                )
                            nc.vector.tensor_add(
                                ot[:, oh * 512 : (oh + 1) * 512],
                                ps[:],
                                bo_bc[:, oh * 512 : (oh + 1) * 512],
                            )
                        nc.sync.dma_start(our[tt], ot[:])

    return _split_waits(nc) if split_waits else nc


def _prep_in_maps(inputs):
    q = np.asarray(inputs["query"], np.float32)
    k = np.asarray(inputs["key"], np.float32)
    v = np.asarray(inputs["value"], np.float32)
    mask = np.asarray(inputs["mask"])
    Wq = np.asarray(inputs["Wq"], np.float32)
    Wk = np.asarray(inputs["Wk"], np.float32)
    Wv = np.asarray(inputs["Wv"], np.float32)
    Wo = np.asarray(inputs["Wo"], np.float32)
    bq = np.asarray(inputs["bq"], np.float32)
    bk = np.asarray(inputs["bk"], np.float32)
    bv = np.asarray(inputs["bv"], np.float32)
    bo = np.asarray(inputs["bo"], np.float32)

    masked = not bool((mask != 0).all())
    xT = {}
    for nm, x in (("q", q), ("k", k), ("v", v)):
        for b in range(B):
            xT[(nm, b)] = np.ascontiguousarray(x[b].T).astype(bf16)
    if masked:
        maskT = np.ascontiguousarray(
            (np.broadcast_to(mask[0, 0], (S, S)).T != 0)
        ).astype(bf16)

    in_maps = []
    for c in range(NCORES):
        b, hg = c // HG, c % HG
        sl = slice(hg * OC, (hg + 1) * OC)
        m = {
            "xqT": xT[("q", b)],
            "xkT": xT[("k", b)],
            "xvT": xT[("v", b)],
            "wqT": np.ascontiguousarray(Wq[sl].T).astype(bf16),
            "wkT": np.ascontiguousarray(Wk[sl].T).astype(bf16),
            "wvT": np.ascontiguousarray(Wv[sl].T).astype(bf16),
            "bq2": np.ascontiguousarray(bq[sl].reshape(2, 128).T),
            "bk2": np.ascontiguousarray(bk[sl].reshape(2, 128).T),
            "bvr": bv[sl].reshape(1, OC).copy(),
            "woR": np.ascontiguousarray(Wo[:, sl].T).astype(bf16),
            "bor": (bo if hg == 0 else np.zeros_like(bo)).reshape(1, D).copy(),
        }
        if masked:
            m["maskT"] = maskT
        in_maps.append(m)
    return in_maps, masked


def _install_profile_hook():
    """Provide antenv.axon_hooks + register the NTFF profile hook via ctypes
    against libaxon_pjrt.so (the agent image lacks antenv.axon_hooks, which
    makes run_bass_kernel_spmd(trace=True) fall over; see trn_boot.py)."""
    import types
    import ctypes
    import contextlib

    if "antenv.axon_hooks" in sys.modules:
        return
    mod = types.ModuleType("antenv.axon_hooks")
    state = {"hook": None}
    mod.set_axon_ntff_profile_hook = lambda h: state.__setitem__("hook", h)
    mod.get_axon_ntff_profile_hook = lambda: state["hook"]
    sys.modules["antenv.axon_hooks"] = mod

    so_path = "/opt/axon/libaxon_pjrt.so"
    if not os.path.exists(so_path):
        return
    lib = ctypes.CDLL(so_path)
    if not hasattr(lib, "axon_start_nrt_profile"):
        return
    lib.axon_start_nrt_profile.argtypes = [
        ctypes.POINTER(ctypes.c_int64),
        ctypes.c_size_t,
    ]
    lib.axon_start_nrt_profile.restype = ctypes.c_int64
    lib.axon_stop_nrt_profile.argtypes = [ctypes.c_char_p]
    lib.axon_stop_nrt_profile.restype = ctypes.c_int64

    @contextlib.contextmanager
    def _hook(output_dir, device_ids):
        import jax

        jax.devices()
        if device_ids:
            ids = (ctypes.c_int64 * len(device_ids))(*device_ids)
            rc = lib.axon_start_nrt_profile(ids, len(device_ids))
        else:
            rc = lib.axon_start_nrt_profile(None, 0)
        if rc != 0:
            raise RuntimeError(f"axon_start_nrt_profile rc={rc}")
        try:
            yield
        finally:
            n = lib.axon_stop_nrt_profile(str(output_dir).encode())
            print(f"profile: {n} file(s) written to {output_dir}", file=sys.stderr)

    mod.set_axon_ntff_profile_hook(_hook)


def run(inputs, trace=False):
    if trace:
        _install_profile_hook()
    in_maps, masked = _prep_in_maps(inputs)
    nc = _build(masked)
    res = bass_utils.run_bass_kernel_spmd(
        nc, in_maps, core_ids=list(range(NCORES)), trace=trace
    )
    out = np.zeros((B, S, D), np.float32)
    for c in range(NCORES):
        out[c // HG] += res.results[c]["out"]
    return out, res


def kernel(**inputs):
    return run(inputs, trace=False)[0]


# revision 4
# speedup vs baseline: 1.5579x; 1.0460x over previous
"""Trainium2 Bass kernel for nn_MultiHeadAttention (B=2, S=4096, D=1024, H=16, Dh=64).

Sharding over 8 cores: core c handles batch b=c//4 and head-group hg=c%4
(4 heads = 256 channels). Host gathers by summing the 4 per-head-group partial
output projections per batch (row-parallel output projection).

Per-core device pipeline (all matmuls bf16, fp32 PSUM accumulation), built
around keeping the ACT (scalar) engine 100% busy on the softmax exp — the
hard floor for this problem (~560us of exp at 1 elem/cycle/lane).

q-blocks of 512 tokens; per (qb, pair-of-heads) block, per k-tile kt (128 keys):
  QK^T: 2 matmuls (one per head), lhsT=KT[64d,128k] rhs=QT[64d,512q]
        -> ST [128k, 2*512q] psum; the two heads use disjoint PE row groups
        (K=64 at row offset 0/64) and run concurrently (~126ns/MM measured).
  exp:  one ACT instruction over the whole [128,1024] ST tile -> ET bf16.
  AV:   2 matmuls (one per head), lhsT=V[128k,64d] rhs=ET[128k,512q]
        accumulated into attn psum [2*64d, 512q]; disjoint PE col groups,
        concurrent. start/stop via kt==0/31 (no zero-fill pass).
  rowsum: DVE bf16 halving tree over the 32 ET k-planes.
Block end: ones-matmul reduces+broadcasts rowsums, DVE reciprocal,
normalize attn -> attnT (bf16). Out-projection per 512-token q-block:
out[t,o] = sum_c attnT[c,t] WoR[c,o] + bo, streamed to HBM.
"""

import math
import os
import sys
import functools

import numpy as np
import ml_dtypes

sys.path.insert(0, "/opt/trn_rl_repo")

import concourse.bass as bass  # noqa: E402
import concourse.mybir as mybir  # noqa: E402
import concourse.tile as tile  # noqa: E402
from concourse import bass_utils  # noqa: E402

B, S, D, H, DH = 2, 4096, 1024, 16, 64
NCORES = 8
HG = 4  # head groups (cores per batch)
OC = 256  # q/k/v channels per core
BF16 = mybir.dt.bfloat16
F32 = mybir.dt.float32
QBLK = 512
NQB = S // QBLK  # 8
NKT = S // 128  # 32 k-tiles
bf16 = ml_dtypes.bfloat16


_TPB_ENGINES = None


def _split_waits(nc, max_waits=1):
    """walrus codegen in this container rejects TPB instructions carrying more
    than one sync-wait command.  Spill extra semaphore waits onto preceding
    NoOps on the same engine (engines execute their queue in order, so a NoOp
    that waits immediately before the instruction is equivalent)."""
    import bass_rust

    global _TPB_ENGINES
    if _TPB_ENGINES is None:
        _TPB_ENGINES = {
            mybir.EngineType.Pool,
            mybir.EngineType.Activation,
            mybir.EngineType.PE,
            mybir.EngineType.DVE,
            mybir.EngineType.SP,
        }
    ctr = 0
    for bb in nc.main_func.blocks:
        insts = bb.instructions
        out = []
        changed = False
        for inst in insts:
            si = getattr(inst, "sync_info", None)
            if (
                si is not None
                and si.on_wait
                and len(si.on_wait) > max_waits
                and inst.engine in _TPB_ENGINES
            ):
                waits = list(si.on_wait)
                keep = waits[-max_waits:]
                spill = waits[:-max_waits]
                for i in range(0, len(spill), max_waits):
                    nop = bass_rust.InstNoOp(
                        name=f"{inst.name}-sw{ctr}", ins=[], outs=[]
                    )
                    ctr += 1
                    nop.engine = inst.engine
                    nop.sync_info = mybir.SyncInfo(
                        on_wait=spill[i : i + max_waits], on_update=[]
                    )
                    out.append(nop)
                inst.sync_info = mybir.SyncInfo(
                    on_wait=keep, on_update=list(si.on_update)
                )
                changed = True
            out.append(inst)
        if changed:
            insts[:] = out
    return nc


@functools.lru_cache(maxsize=4)
def _build(masked: bool, split_waits: bool = True):
    nc = bass.Bass()

    xqT_d = nc.dram_tensor("xqT", [D, S], BF16, kind="ExternalInput")
    xkT_d = nc.dram_tensor("xkT", [D, S], BF16, kind="ExternalInput")
    xvT_d = nc.dram_tensor("xvT", [D, S], BF16, kind="ExternalInput")
    wqT_d = nc.dram_tensor("wqT", [D, OC], BF16, kind="ExternalInput")
    wkT_d = nc.dram_tensor("wkT", [D, OC], BF16, kind="ExternalInput")
    wvT_d = nc.dram_tensor("wvT", [D, OC], BF16, kind="ExternalInput")
    bq_d = nc.dram_tensor("bq2", [128, 2], F32, kind="ExternalInput")
    bk_d = nc.dram_tensor("bk2", [128, 2], F32, kind="ExternalInput")
    bv_d = nc.dram_tensor("bvr", [1, OC], F32, kind="ExternalInput")
    woR_d = nc.dram_tensor("woR", [OC, D], BF16, kind="ExternalInput")
    bo_d = nc.dram_tensor("bor", [1, D], F32, kind="ExternalInput")
    if masked:
        maskT_d = nc.dram_tensor("maskT", [S, S], BF16, kind="ExternalInput")
    out_d = nc.dram_tensor("out", [S, D], F32, kind="ExternalOutput")

    with tile.TileContext(nc) as tc:
        with (
            tc.tile_pool(name="persist", bufs=1) as persist,
            tc.tile_pool(name="wpool", bufs=1) as wpool,
            tc.tile_pool(name="xt", bufs=2) as xtp,
            tc.tile_pool(name="et", bufs=5) as etp,
            tc.tile_pool(name="rtmp", bufs=3) as rtmp,
            tc.tile_pool(name="rrow", bufs=2) as rrow,
            tc.tile_pool(name="outp", bufs=2) as outp,
            tc.tile_pool(name="stp", bufs=2, space="PSUM") as stp,
            tc.tile_pool(name="avp", bufs=2, space="PSUM") as avp,
            tc.tile_pool(name="misc", bufs=2, space="PSUM") as miscp,
        ):
            # persistent SBUF tensors
            QT = [persist.tile([128, S], BF16, tag=f"QT{p}", name=f"QT{p}") for p in range(2)]
            KT = [persist.tile([128, S], BF16, tag=f"KT{p}", name=f"KT{p}") for p in range(2)]
            Vsb = persist.tile([128, NKT, OC], BF16, tag="Vsb")
            attnT = [persist.tile([128, S], BF16, tag=f"attnT{p}", name=f"attnT{p}") for p in range(2)]
            ones_row = persist.tile([1, 128], F32, tag="ones_row")
            ones64 = persist.tile([128, 64], BF16, tag="ones64")
            bv_bc = persist.tile([128, OC], F32, tag="bv_bc")
            bo_bc = persist.tile([128, D], F32, tag="bo_bc")
            nc.gpsimd.memset(ones_row[:], 1.0)
            nc.gpsimd.memset(ones64[:], 1.0)

            wq_sb = wpool.tile([128, 8, OC], BF16, tag="wq")
            wk_sb = wpool.tile([128, 8, OC], BF16, tag="wk")
            wv_sb = wpool.tile([128, 8, OC], BF16, tag="wv")
            wo_sb = wpool.tile([128, 2, D], BF16, tag="wo")
            bq_sb = wpool.tile([128, 2], F32, tag="bq")
            bk_sb = wpool.tile([128, 2], F32, tag="bk")
            bv_sb = wpool.tile([1, OC], F32, tag="bv")
            bo_sb = wpool.tile([1, D], F32, tag="bo")
            nc.sync.dma_start(wq_sb[:], wqT_d.rearrange("(dc p) o -> p dc o", p=128))
            nc.sync.dma_start(wk_sb[:], wkT_d.rearrange("(dc p) o -> p dc o", p=128))
            nc.sync.dma_start(wv_sb[:], wvT_d.rearrange("(dc p) o -> p dc o", p=128))
            nc.sync.dma_start(wo_sb[:], woR_d.rearrange("(cc p) o -> p cc o", p=128))
            nc.sync.dma_start(bq_sb[:], bq_d[:])
            nc.sync.dma_start(bk_sb[:], bk_d[:])
            nc.sync.dma_start(bv_sb[:], bv_d[:])
            nc.sync.dma_start(bo_sb[:], bo_d[:])

            # warm the ACT exp table while projections run (off critical path)
            warm = rtmp.tile([1, 2], F32, tag="warm", name="warm")
            nc.scalar.activation(
                warm[:], bq_sb[0:1, :], mybir.ActivationFunctionType.Exp
            )

            # broadcast bv / bo across partitions via K=1 matmuls
            ps = miscp.tile([128, 512], F32, tag="ms", name="bvbc")
            nc.tensor.matmul(ps[:, 0:OC], ones_row[:], bv_sb[:], start=True, stop=True)
            nc.vector.tensor_copy(bv_bc[:], ps[:, 0:OC])
            for oh in range(2):
                ps = miscp.tile([128, 512], F32, tag="ms", name="bobc")
                nc.tensor.matmul(
                    ps[:],
                    ones_row[:],
                    bo_sb[:, oh * 512 : (oh + 1) * 512],
                    start=True,
                    stop=True,
                )
                nc.vector.tensor_copy(bo_bc[:, oh * 512 : (oh + 1) * 512], ps[:])

            # ---------------- Q/K projections ----------------
            for which, (xd, wsb, bsb, dst) in enumerate(
                (
                    (xkT_d, wk_sb, bk_sb, KT),
                    (xqT_d, wq_sb, bq_sb, QT),
                )
            ):
                xr = xd.rearrange("(dc p) t -> p dc t", p=128)
                for tt in range(8):  # 512-token tiles
                    # Q-proj beyond the first q-block defers into PE gaps so
                    # the attention exp stream starts as early as possible.
                    defer = which == 1 and tt >= 1
                    with tc.high_priority(offset=-1000 if defer else 0):
                        xt = xtp.tile([128, 8, 512], BF16, tag="xqk", name="xqk")
                        nc.sync.dma_start(xt[:], xr[:, :, tt * 512 : (tt + 1) * 512])
                        for oc in range(2):
                            pst = miscp.tile([128, 512], F32, tag="ms", name="pqk")
                            for dc in range(8):
                                nc.tensor.matmul(
                                    pst[:],
                                    wsb[:, dc, oc * 128 : (oc + 1) * 128],
                                    xt[:, dc, :],
                                    start=(dc == 0),
                                    stop=(dc == 7),
                                )
                            nc.vector.tensor_scalar_add(
                                dst[oc][:, tt * 512 : (tt + 1) * 512],
                                pst[:],
                                bsb[:, oc : oc + 1],
                            )

            # ---------------- V projection (fills PE gaps during attention) --
            xvr = xvT_d.rearrange("(dc p) t -> p dc t", p=128)
            with tc.high_priority(offset=-2000):
                for tcI in range(NKT):
                    xt = xtp.tile([128, 8, 128], BF16, tag="xv", name="xv")
                    nc.sync.dma_start(xt[:], xvr[:, :, tcI * 128 : (tcI + 1) * 128])
                    pst = miscp.tile([128, 512], F32, tag="ms", name="pv")
                    for dc in range(8):
                        nc.tensor.matmul(
                            pst[:, 0:OC],
                            xt[:, dc, :],
                            wv_sb[:, dc, :],
                            start=(dc == 0),
                            stop=(dc == 7),
                        )
                    nc.vector.tensor_add(Vsb[:, tcI, :], pst[:, 0:OC], bv_bc[:])

            # ---------------- attention + output projection ----------------
            if masked:
                mrr = maskT_d.rearrange("(kt p) q -> kt p q", p=128)
            our = out_d.rearrange("(tt p) o -> tt p o", p=128)
            for qb in range(NQB):
                q0 = qb * QBLK
                for pair in range(2):
                    QTp, KTp, ATp = QT[pair], KT[pair], attnT[pair]
                    attn_ps = avp.tile([128, QBLK], F32, tag="attn", name="attn")
                    rs5 = None
                    et = None
                    for kt in range(NKT):
                        ki = kt % 4
                        if ki == 0:
                            et = etp.tile([128, 4, 1024], BF16, tag="et", name="et")
                        st = stp.tile([128, 1024], F32, tag="st", name="st")
                        for h2 in range(2):
                            b0 = h2 * 64
                            nc.tensor.matmul(
                                st[:, h2 * 512 : (h2 + 1) * 512],
                                KTp[b0 : b0 + 64, kt * 128 : (kt + 1) * 128],
                                QTp[b0 : b0 + 64, q0 : q0 + QBLK],
                                start=True,
                                stop=True,
                            )
                        nc.scalar.activation(
                            et[:, ki, :],
                            st[:],
                            mybir.ActivationFunctionType.Exp,
                            scale=1.0 / math.sqrt(DH),
                        )
                        if masked:
                            mk = rtmp.tile([128, 512], BF16, tag="mk", name="mk")
                            nc.sync.dma_start(mk[:], mrr[kt][:, q0 : q0 + QBLK])
                            for h2 in range(2):
                                nc.vector.tensor_mul(
                                    et[:, ki, h2 * 512 : (h2 + 1) * 512],
                                    et[:, ki, h2 * 512 : (h2 + 1) * 512],
                                    mk[:],
                                )
                        # AV accumulate (deprioritized: fills PE gaps)
                        with tc.high_priority(offset=-8000):
                            for h2 in range(2):
                                b0 = h2 * 64
                                h_local = pair * 2 + h2
                                nc.tensor.matmul(
                                    attn_ps[b0 : b0 + 64, :],
                                    Vsb[:, kt, h_local * 64 : (h_local + 1) * 64],
                                    et[:, ki, h2 * 512 : (h2 + 1) * 512],
                                    start=(kt == 0),
                                    stop=(kt == NKT - 1),
                                    skip_group_check=True,
                                )
                        # rowsum tree contribution once per 4-k-tile group
                        if ki == 3:
                            t2 = rtmp.tile([128, 2, 1024], BF16, tag="t2", name="t2")
                            nc.vector.tensor_add(t2[:], et[:, 0:2, :], et[:, 2:4, :])
                            if kt == 3:
                                rs5 = rrow.tile([128, 1024], BF16, tag="rs5", name="rs5")
                                nc.vector.tensor_add(rs5[:], t2[:, 0, :], t2[:, 1, :])
                            else:
                                ts_ = rtmp.tile([128, 1024], BF16, tag="ts", name="ts")
                                nc.vector.tensor_add(ts_[:], t2[:, 0, :], t2[:, 1, :])
                                nc.vector.tensor_add(rs5[:], rs5[:], ts_[:])
                    # block end: rowsum reduce+broadcast, reciprocal, normalize
                    with tc.high_priority(offset=-8000):
                        st_rs = miscp.tile([128, 512], F32, tag="ms", name="st_rs")
                        for h2 in range(2):
                            nc.tensor.matmul(
                                st_rs[h2 * 64 : h2 * 64 + 64, :],
                                ones64[:],
                                rs5[:, h2 * 512 : (h2 + 1) * 512],
                                start=True,
                                stop=True,
                            )
                        rs_bc = rrow.tile([128, 512], F32, tag="rsbc", name="rsbc")
                        nc.vector.reciprocal(rs_bc[:], st_rs[:])
                        nc.vector.tensor_tensor(
                            ATp[:, q0 : q0 + QBLK],
                            attn_ps[:],
                            rs_bc[:],
                            mybir.AluOpType.mult,
                        )
                # output projection for this q-block's token tiles
                with tc.high_priority(offset=-8000):
                    for tt in range(qb * 4, (qb + 1) * 4):
                        ot = outp.tile([128, D], F32, tag="ot", name="ot")
                        for oh in range(2):
                            ps = miscp.tile([128, 512], F32, tag="ms", name="po")
                            for cc in range(2):
                                nc.tensor.matmul(
                                    ps[:],
                                    attnT[cc][:, tt * 128 : (tt + 1) * 128],
                                    wo_sb[:, cc, oh * 512 : (oh + 1) * 512],
                                    start=(cc == 0),
                                    stop=(cc == 1),
                                )
                            nc.vector.tensor_add(
                                ot[:, oh * 512 : (oh + 1) * 512],
                                ps[:],
                                bo_bc[:, oh * 512 : (oh + 1) * 512],
                            )
                        nc.sync.dma_start(our[tt], ot[:])

    return _split_waits(nc) if split_waits else nc


def _prep_in_maps(inputs):
    q = np.asarray(inputs["query"], np.float32)
    k = np.asarray(inputs["key"], np.float32)
    v = np.asarray(inputs["value"], np.float32)
    mask = np.asarray(inputs["mask"])
    Wq = np.asarray(inputs["Wq"], np.float32)
    Wk = np.asarray(inputs["Wk"], np.float32)
    Wv = np.asarray(inputs["Wv"], np.float32)
    Wo = np.asarray(inputs["Wo"], np.float32)
    bq = np.asarray(inputs["bq"], np.float32)
    bk = np.asarray(inputs["bk"], np.float32)
    bv = np.asarray(inputs["bv"], np.float32)
    bo = np.asarray(inputs["bo"], np.float32)

    masked = not bool((mask != 0).all())
    xT = {}
    for nm, x in (("q", q), ("k", k), ("v", v)):
        for b in range(B):
            xT[(nm, b)] = np.ascontiguousarray(x[b].T).astype(bf16)
    if masked:
        maskT = np.ascontiguousarray(
            (np.broadcast_to(mask[0, 0], (S, S)).T != 0)
        ).astype(bf16)

    in_maps = []
    for c in range(NCORES):
        b, hg = c // HG, c % HG
        sl = slice(hg * OC, (hg + 1) * OC)
        m = {
            "xqT": xT[("q", b)],
            "xkT": xT[("k", b)],
            "xvT": xT[("v", b)],
            "wqT": np.ascontiguousarray(Wq[sl].T).astype(bf16),
            "wkT": np.ascontiguousarray(Wk[sl].T).astype(bf16),
            "wvT": np.ascontiguousarray(Wv[sl].T).astype(bf16),
            "bq2": np.ascontiguousarray(bq[sl].reshape(2, 128).T),
            "bk2": np.ascontiguousarray(bk[sl].reshape(2, 128).T),
            "bvr": bv[sl].reshape(1, OC).copy(),
            "woR": np.ascontiguousarray(Wo[:, sl].T).astype(bf16),
            "bor": (bo if hg == 0 else np.zeros_like(bo)).reshape(1, D).copy(),
        }
        if masked:
            m["maskT"] = maskT
        in_maps.append(m)
    return in_maps, masked


def _install_profile_hook():
    """Provide antenv.axon_hooks + register the NTFF profile hook via ctypes
    against libaxon_pjrt.so (the agent image lacks antenv.axon_hooks, which
    makes run_bass_kernel_spmd(trace=True) fall over; see trn_boot.py)."""
    import types
    import ctypes
    import contextlib

    if "antenv.axon_hooks" in sys.modules:
        return
    mod = types.ModuleType("antenv.axon_hooks")
    state = {"hook": None}
    mod.set_axon_ntff_profile_hook = lambda h: state.__setitem__("hook", h)
    mod.get_axon_ntff_profile_hook = lambda: state["hook"]
    sys.modules["antenv.axon_hooks"] = mod

    so_path = "/opt/axon/libaxon_pjrt.so"
    if not os.path.exists(so_path):
        return
    lib = ctypes.CDLL(so_path)
    if not hasattr(lib, "axon_start_nrt_profile"):
        return
    lib.axon_start_nrt_profile.argtypes = [
        ctypes.POINTER(ctypes.c_int64),
        ctypes.c_size_t,
    ]
    lib.axon_start_nrt_profile.restype = ctypes.c_int64
    lib.axon_stop_nrt_profile.argtypes = [ctypes.c_char_p]
    lib.axon_stop_nrt_profile.restype = ctypes.c_int64

    @contextlib.contextmanager
    def _hook(output_dir, device_ids):
        import jax

        jax.devices()
        if device_ids:
            ids = (ctypes.c_int64 * len(device_ids))(*device_ids)
            rc = lib.axon_start_nrt_profile(ids, len(device_ids))
        else:
            rc = lib.axon_start_nrt_profile(None, 0)
        if rc != 0:
            raise RuntimeError(f"axon_start_nrt_profile rc={rc}")
        try:
            yield
        finally:
            n = lib.axon_stop_nrt_profile(str(output_dir).encode())
            print(f"profile: {n} file(s) written to {output_dir}", file=sys.stderr)

    mod.set_axon_ntff_profile_hook(_hook)


def run(inputs, trace=False):
    if trace:
        _install_profile_hook()
    in_maps, masked = _prep_in_maps(inputs)
    nc = _build(masked)
    res = bass_utils.run_bass_kernel_spmd(
        nc, in_maps, core_ids=list(range(NCORES)), trace=trace
    )
    out = np.zeros((B, S, D), np.float32)
    for c in range(NCORES):
        out[c // HG] += res.results[c]["out"]
    return out, res


def kernel(**inputs):
    return run(inputs, trace=False)[0]


# revision 8
# speedup vs baseline: 1.6105x; 1.0337x over previous
"""Trainium2 Bass kernel for nn_MultiHeadAttention (B=2, S=4096, D=1024, H=16, Dh=64).

Sharding over 8 cores: core c handles batch b=c//4 and head-group hg=c%4
(4 heads = 256 channels). Host gathers by summing the 4 per-head-group partial
output projections per batch (row-parallel output projection).

Per-core device pipeline (all matmuls bf16, fp32 PSUM accumulation), built
around keeping the ACT (scalar) engine 100% busy on the softmax exp — the
hard floor for this problem (~560us of exp at 1 elem/cycle/lane).

q-blocks of 512 tokens; per (qb, pair-of-heads) block, per k-tile kt (128 keys):
  QK^T: 2 matmuls (one per head), lhsT=KT[64d,128k] rhs=QT[64d,512q]
        -> ST [128k, 2*512q] psum; the two heads use disjoint PE row groups
        (K=64 at row offset 0/64) and run concurrently (~126ns/MM measured).
  exp:  one ACT instruction over the whole [128,1024] ST tile -> ET bf16.
  AV:   2 matmuls (one per head), lhsT=V[128k,64d] rhs=ET[128k,512q]
        accumulated into attn psum [2*64d, 512q]; disjoint PE col groups,
        concurrent. start/stop via kt==0/31 (no zero-fill pass).
  rowsum: DVE bf16 halving tree over the 32 ET k-planes.
Block end: ones-matmul reduces+broadcasts rowsums, DVE reciprocal,
normalize attn -> attnT (bf16). Out-projection per 512-token q-block:
out[t,o] = sum_c attnT[c,t] WoR[c,o] + bo, streamed to HBM.
"""

import math
import os
import sys
import functools

import numpy as np
import ml_dtypes

sys.path.insert(0, "/opt/trn_rl_repo")

import concourse.bass as bass  # noqa: E402
import concourse.mybir as mybir  # noqa: E402
import concourse.tile as tile  # noqa: E402
from concourse import bass_utils  # noqa: E402

B, S, D, H, DH = 2, 4096, 1024, 16, 64
NCORES = 8
HG = 4  # head groups (cores per batch)
OC = 256  # q/k/v channels per core
BF16 = mybir.dt.bfloat16
F32 = mybir.dt.float32
QBLK = 512
NQB = S // QBLK  # 8
NKT = S // 128  # 32 k-tiles
bf16 = ml_dtypes.bfloat16


_TPB_ENGINES = None


def _split_waits(nc, max_waits=1):
    """walrus codegen in this container rejects TPB instructions carrying more
    than one sync-wait command.  Spill extra semaphore waits onto preceding
    NoOps on the same engine (engines execute their queue in order, so a NoOp
    that waits immediately before the instruction is equivalent)."""
    import bass_rust

    global _TPB_ENGINES
    if _TPB_ENGINES is None:
        _TPB_ENGINES = {
            mybir.EngineType.Pool,
            mybir.EngineType.Activation,
            mybir.EngineType.PE,
            mybir.EngineType.DVE,
            mybir.EngineType.SP,
        }
    ctr = 0
    for bb in nc.main_func.blocks:
        insts = bb.instructions
        out = []
        changed = False
        for inst in insts:
            si = getattr(inst, "sync_info", None)
            if (
                si is not None
                and si.on_wait
                and len(si.on_wait) > max_waits
                and inst.engine in _TPB_ENGINES
            ):
                waits = list(si.on_wait)
                keep = waits[-max_waits:]
                spill = waits[:-max_waits]
                for i in range(0, len(spill), max_waits):
                    nop = bass_rust.InstNoOp(
                        name=f"{inst.name}-sw{ctr}", ins=[], outs=[]
                    )
                    ctr += 1
                    nop.engine = inst.engine
                    nop.sync_info = mybir.SyncInfo(
                        on_wait=spill[i : i + max_waits], on_update=[]
                    )
                    out.append(nop)
                inst.sync_info = mybir.SyncInfo(
                    on_wait=keep, on_update=list(si.on_update)
                )
                changed = True
            out.append(inst)
        if changed:
            insts[:] = out
    return nc


@functools.lru_cache(maxsize=4)
def _build(masked: bool, split_waits: bool = True):
    nc = bass.Bass()

    xqT_d = nc.dram_tensor("xqT", [D, S], BF16, kind="ExternalInput")
    xkT_d = nc.dram_tensor("xkT", [D, S], BF16, kind="ExternalInput")
    xvT_d = nc.dram_tensor("xvT", [D, S], BF16, kind="ExternalInput")
    wqT_d = nc.dram_tensor("wqT", [D, OC], BF16, kind="ExternalInput")
    wkT_d = nc.dram_tensor("wkT", [D, OC], BF16, kind="ExternalInput")
    wvT_d = nc.dram_tensor("wvT", [D, OC], BF16, kind="ExternalInput")
    bq_d = nc.dram_tensor("bq2", [128, 2], F32, kind="ExternalInput")
    bk_d = nc.dram_tensor("bk2", [128, 2], F32, kind="ExternalInput")
    bv_d = nc.dram_tensor("bvr", [1, OC], F32, kind="ExternalInput")
    woR_d = nc.dram_tensor("woR", [OC, D], BF16, kind="ExternalInput")
    bo_d = nc.dram_tensor("bor", [1, D], F32, kind="ExternalInput")
    if masked:
        maskT_d = nc.dram_tensor("maskT", [S, S], BF16, kind="ExternalInput")
    out_d = nc.dram_tensor("out", [S, D], F32, kind="ExternalOutput")

    with tile.TileContext(nc) as tc:
        with (
            tc.tile_pool(name="persist", bufs=1) as persist,
            tc.tile_pool(name="wpool", bufs=1) as wpool,
            tc.tile_pool(name="xt", bufs=2) as xtp,
            tc.tile_pool(name="et", bufs=4) as etp,
            tc.tile_pool(name="rtmp", bufs=3) as rtmp,
            tc.tile_pool(name="rrow", bufs=2) as rrow,
            tc.tile_pool(name="outp", bufs=2) as outp,
            tc.tile_pool(name="stp", bufs=2, space="PSUM") as stp,
            tc.tile_pool(name="avp", bufs=2, space="PSUM") as avp,
            tc.tile_pool(name="misc", bufs=2, space="PSUM") as miscp,
        ):
            # persistent SBUF tensors
            QT = [persist.tile([128, S], BF16, tag=f"QT{p}", name=f"QT{p}") for p in range(2)]
            KT = [persist.tile([128, S], BF16, tag=f"KT{p}", name=f"KT{p}") for p in range(2)]
            Vsb = persist.tile([128, NKT, OC], BF16, tag="Vsb")
            attnT = [persist.tile([128, S], BF16, tag=f"attnT{p}", name=f"attnT{p}") for p in range(2)]
            ones_row = persist.tile([1, 128], F32, tag="ones_row")
            ones64 = persist.tile([128, 64], BF16, tag="ones64")
            bv_bc = persist.tile([128, OC], F32, tag="bv_bc")
            bo_bc = persist.tile([128, D], F32, tag="bo_bc")
            nc.gpsimd.memset(ones_row[:], 1.0)
            nc.gpsimd.memset(ones64[:], 1.0)

            wq_sb = wpool.tile([128, 8, OC], BF16, tag="wq")
            wk_sb = wpool.tile([128, 8, OC], BF16, tag="wk")
            wv_sb = wpool.tile([128, 8, OC], BF16, tag="wv")
            wo_sb = wpool.tile([128, 2, D], BF16, tag="wo")
            bq_sb = wpool.tile([128, 2], F32, tag="bq")
            bk_sb = wpool.tile([128, 2], F32, tag="bk")
            bv_sb = wpool.tile([1, OC], F32, tag="bv")
            bo_sb = wpool.tile([1, D], F32, tag="bo")
            nc.sync.dma_start(wq_sb[:], wqT_d.rearrange("(dc p) o -> p dc o", p=128))
            nc.sync.dma_start(wk_sb[:], wkT_d.rearrange("(dc p) o -> p dc o", p=128))
            nc.sync.dma_start(wv_sb[:], wvT_d.rearrange("(dc p) o -> p dc o", p=128))
            nc.sync.dma_start(wo_sb[:], woR_d.rearrange("(cc p) o -> p cc o", p=128))
            nc.sync.dma_start(bq_sb[:], bq_d[:])
            nc.sync.dma_start(bk_sb[:], bk_d[:])
            nc.sync.dma_start(bv_sb[:], bv_d[:])
            nc.sync.dma_start(bo_sb[:], bo_d[:])

            # warm the ACT exp table while projections run (off critical path)
            warm = rtmp.tile([1, 2], F32, tag="warm", name="warm")
            nc.scalar.activation(
                warm[:], bq_sb[0:1, :], mybir.ActivationFunctionType.Exp
            )

            # broadcast bv / bo across partitions via K=1 matmuls
            ps = miscp.tile([128, 512], F32, tag="ms", name="bvbc")
            nc.tensor.matmul(ps[:, 0:OC], ones_row[:], bv_sb[:], start=True, stop=True)
            nc.vector.tensor_copy(bv_bc[:], ps[:, 0:OC])
            for oh in range(2):
                ps = miscp.tile([128, 512], F32, tag="ms", name="bobc")
                nc.tensor.matmul(
                    ps[:],
                    ones_row[:],
                    bo_sb[:, oh * 512 : (oh + 1) * 512],
                    start=True,
                    stop=True,
                )
                nc.vector.tensor_copy(bo_bc[:, oh * 512 : (oh + 1) * 512], ps[:])

            # ---------------- projections ----------------
            # K(tt0)+Q(tt0) first at full priority so the attention exp
            # stream starts immediately; remaining token tiles fill PE gaps
            # under the stream (Tile tracks slice-level deps on QT/KT).
            xkr = xkT_d.rearrange("(dc p) t -> p dc t", p=128)
            xqr = xqT_d.rearrange("(dc p) t -> p dc t", p=128)

            def kq_proj(xr, wsb, bsb, dst, tt):
                xt = xtp.tile([128, 8, 512], BF16, tag="xqk", name="xqk")
                nc.sync.dma_start(xt[:], xr[:, :, tt * 512 : (tt + 1) * 512])
                for oc in range(2):
                    pst = miscp.tile([128, 512], F32, tag="ms", name="pqk")
                    for dc in range(8):
                        nc.tensor.matmul(
                            pst[:],
                            wsb[:, dc, oc * 128 : (oc + 1) * 128],
                            xt[:, dc, :],
                            start=(dc == 0),
                            stop=(dc == 7),
                        )
                    nc.vector.tensor_scalar_add(
                        dst[oc][:, tt * 512 : (tt + 1) * 512],
                        pst[:],
                        bsb[:, oc : oc + 1],
                    )

            kq_proj(xkr, wk_sb, bk_sb, KT, 0)
            kq_proj(xqr, wq_sb, bq_sb, QT, 0)
            for tt in range(1, 8):
                with tc.high_priority(offset=-500):
                    kq_proj(xkr, wk_sb, bk_sb, KT, tt)
            for tt in range(1, 8):
                with tc.high_priority(offset=-1000):
                    kq_proj(xqr, wq_sb, bq_sb, QT, tt)

            # V projection: big DMA chunks (2KB descriptors), fills PE gaps
            xvr = xvT_d.rearrange("(dc p) t -> p dc t", p=128)
            with tc.high_priority(offset=-2000):
                for vt in range(4):
                    xt = xtp.tile([128, 8, 1024], BF16, tag="xv", name="xv")
                    nc.sync.dma_start(xt[:], xvr[:, :, vt * 1024 : (vt + 1) * 1024])
                    for sub in range(8):
                        tcI = vt * 8 + sub
                        pst = miscp.tile([128, 512], F32, tag="ms", name="pv")
                        for dc in range(8):
                            nc.tensor.matmul(
                                pst[:, 0:OC],
                                xt[:, dc, sub * 128 : (sub + 1) * 128],
                                wv_sb[:, dc, :],
                                start=(dc == 0),
                                stop=(dc == 7),
                            )
                        nc.vector.tensor_add(Vsb[:, tcI, :], pst[:, 0:OC], bv_bc[:])

            # ---------------- attention + output projection ----------------
            if masked:
                mrr = maskT_d.rearrange("(kt p) q -> kt p q", p=128)
            our = out_d.rearrange("(tt p) o -> tt p o", p=128)
            def out_proj(qb):
                # output projection for q-block qb's token tiles
                with tc.high_priority(offset=-8000):
                    for tt in range(qb * 4, (qb + 1) * 4):
                        ot = outp.tile([128, D], F32, tag="ot", name="ot")
                        for oh in range(2):
                            ps = miscp.tile([128, 512], F32, tag="ms", name="po")
                            for cc in range(2):
                                nc.tensor.matmul(
                                    ps[:],
                                    attnT[cc][:, tt * 128 : (tt + 1) * 128],
                                    wo_sb[:, cc, oh * 512 : (oh + 1) * 512],
                                    start=(cc == 0),
                                    stop=(cc == 1),
                                )
                            nc.vector.tensor_add(
                                ot[:, oh * 512 : (oh + 1) * 512],
                                ps[:],
                                bo_bc[:, oh * 512 : (oh + 1) * 512],
                            )
                        nc.sync.dma_start(our[tt], ot[:])

            for qb in range(NQB):
                q0 = qb * QBLK
                for pair in range(2):
                    QTp, KTp, ATp = QT[pair], KT[pair], attnT[pair]
                    attn_ps = avp.tile([128, QBLK], F32, tag="attn", name="attn")
                    rs5 = None
                    et = None
                    for kt in range(NKT):
                        ki = kt % 4
                        if ki == 0:
                            et = etp.tile([128, 4, 1024], BF16, tag="et", name="et")
                        st = stp.tile([128, 1024], F32, tag="st", name="st")
                        for h2 in range(2):
                            b0 = h2 * 64
                            nc.tensor.matmul(
                                st[:, h2 * 512 : (h2 + 1) * 512],
                                KTp[b0 : b0 + 64, kt * 128 : (kt + 1) * 128],
                                QTp[b0 : b0 + 64, q0 : q0 + QBLK],
                                start=True,
                                stop=True,
                            )
                        nc.scalar.activation(
                            et[:, ki, :],
                            st[:],
                            mybir.ActivationFunctionType.Exp,
                            scale=1.0 / math.sqrt(DH),
                        )
                        if masked:
                            mk = rtmp.tile([128, 512], BF16, tag="mk", name="mk")
                            nc.sync.dma_start(mk[:], mrr[kt][:, q0 : q0 + QBLK])
                            for h2 in range(2):
                                nc.vector.tensor_mul(
                                    et[:, ki, h2 * 512 : (h2 + 1) * 512],
                                    et[:, ki, h2 * 512 : (h2 + 1) * 512],
                                    mk[:],
                                )
                        # AV accumulate (deprioritized: fills PE gaps)
                        with tc.high_priority(offset=-8000):
                            for h2 in range(2):
                                b0 = h2 * 64
                                h_local = pair * 2 + h2
                                nc.tensor.matmul(
                                    attn_ps[b0 : b0 + 64, :],
                                    Vsb[:, kt, h_local * 64 : (h_local + 1) * 64],
                                    et[:, ki, h2 * 512 : (h2 + 1) * 512],
                                    start=(kt == 0),
                                    stop=(kt == NKT - 1),
                                    skip_group_check=True,
                                )
                        # rowsum tree contribution once per 4-k-tile group
                        if ki == 3:
                            t2 = rtmp.tile([128, 2, 1024], BF16, tag="t2", name="t2")
                            nc.vector.tensor_add(t2[:], et[:, 0:2, :], et[:, 2:4, :])
                            if kt == 3:
                                rs5 = rrow.tile([128, 1024], BF16, tag="rs5", name="rs5")
                                nc.vector.tensor_add(rs5[:], t2[:, 0, :], t2[:, 1, :])
                            else:
                                ts_ = rtmp.tile([128, 1024], BF16, tag="ts", name="ts")
                                nc.vector.tensor_add(ts_[:], t2[:, 0, :], t2[:, 1, :])
                                nc.vector.tensor_add(rs5[:], rs5[:], ts_[:])
                    # block end: rowsum reduce+broadcast, reciprocal, normalize
                    with tc.high_priority(offset=-8000):
                        st_rs = miscp.tile([128, 512], F32, tag="ms", name="st_rs")
                        for h2 in range(2):
                            nc.tensor.matmul(
                                st_rs[h2 * 64 : h2 * 64 + 64, :],
                                ones64[:],
                                rs5[:, h2 * 512 : (h2 + 1) * 512],
                                start=True,
                                stop=True,
                            )
                        rs_bc = rrow.tile([128, 512], F32, tag="rsbc", name="rsbc")
                        nc.vector.reciprocal(rs_bc[:], st_rs[:])
                        nc.vector.tensor_tensor(
                            ATp[:, q0 : q0 + QBLK],
                            attn_ps[:],
                            rs_bc[:],
                            mybir.AluOpType.mult,
                        )
                    # previous q-block's output projection is emitted after
                    # this q-block's pair-0 stream so it never head-of-line
                    # blocks the PE queue on the normalize chain.
                    if pair == 0 and qb > 0:
                        out_proj(qb - 1)
            out_proj(NQB - 1)

    return _split_waits(nc) if split_waits else nc


def _prep_in_maps(inputs):
    q = np.asarray(inputs["query"], np.float32)
    k = np.asarray(inputs["key"], np.float32)
    v = np.asarray(inputs["value"], np.float32)
    mask = np.asarray(inputs["mask"])
    Wq = np.asarray(inputs["Wq"], np.float32)
    Wk = np.asarray(inputs["Wk"], np.float32)
    Wv = np.asarray(inputs["Wv"], np.float32)
    Wo = np.asarray(inputs["Wo"], np.float32)
    bq = np.asarray(inputs["bq"], np.float32)
    bk = np.asarray(inputs["bk"], np.float32)
    bv = np.asarray(inputs["bv"], np.float32)
    bo = np.asarray(inputs["bo"], np.float32)

    masked = not bool((mask != 0).all())
    xT = {}
    for nm, x in (("q", q), ("k", k), ("v", v)):
        for b in range(B):
            xT[(nm, b)] = np.ascontiguousarray(x[b].T).astype(bf16)
    if masked:
        maskT = np.ascontiguousarray(
            (np.broadcast_to(mask[0, 0], (S, S)).T != 0)
        ).astype(bf16)

    in_maps = []
    for c in range(NCORES):
        b, hg = c // HG, c % HG
        sl = slice(hg * OC, (hg + 1) * OC)
        m = {
            "xqT": xT[("q", b)],
            "xkT": xT[("k", b)],
            "xvT": xT[("v", b)],
            "wqT": np.ascontiguousarray(Wq[sl].T).astype(bf16),
            "wkT": np.ascontiguousarray(Wk[sl].T).astype(bf16),
            "wvT": np.ascontiguousarray(Wv[sl].T).astype(bf16),
            "bq2": np.ascontiguousarray(bq[sl].reshape(2, 128).T),
            "bk2": np.ascontiguousarray(bk[sl].reshape(2, 128).T),
            "bvr": bv[sl].reshape(1, OC).copy(),
            "woR": np.ascontiguousarray(Wo[:, sl].T).astype(bf16),
            "bor": (bo if hg == 0 else np.zeros_like(bo)).reshape(1, D).copy(),
        }
        if masked:
            m["maskT"] = maskT
        in_maps.append(m)
    return in_maps, masked


def _install_profile_hook():
    """Provide antenv.axon_hooks + register the NTFF profile hook via ctypes
    against libaxon_pjrt.so (the agent image lacks antenv.axon_hooks, which
    makes run_bass_kernel_spmd(trace=True) fall over; see trn_boot.py)."""
    import types
    import ctypes
    import contextlib

    if "antenv.axon_hooks" in sys.modules:
        return
    mod = types.ModuleType("antenv.axon_hooks")
    state = {"hook": None}
    mod.set_axon_ntff_profile_hook = lambda h: state.__setitem__("hook", h)
    mod.get_axon_ntff_profile_hook = lambda: state["hook"]
    sys.modules["antenv.axon_hooks"] = mod

    so_path = "/opt/axon/libaxon_pjrt.so"
    if not os.path.exists(so_path):
        return
    lib = ctypes.CDLL(so_path)
    if not hasattr(lib, "axon_start_nrt_profile"):
        return
    lib.axon_start_nrt_profile.argtypes = [
        ctypes.POINTER(ctypes.c_int64),
        ctypes.c_size_t,
    ]
    lib.axon_start_nrt_profile.restype = ctypes.c_int64
    lib.axon_stop_nrt_profile.argtypes = [ctypes.c_char_p]
    lib.axon_stop_nrt_profile.restype = ctypes.c_int64

    @contextlib.contextmanager
    def _hook(output_dir, device_ids):
        import jax

        jax.devices()
        if device_ids:
            ids = (ctypes.c_int64 * len(device_ids))(*device_ids)
            rc = lib.axon_start_nrt_profile(ids, len(device_ids))
        else:
            rc = lib.axon_start_nrt_profile(None, 0)
        if rc != 0:
            raise RuntimeError(f"axon_start_nrt_profile rc={rc}")
        try:
            yield
        finally:
            n = lib.axon_stop_nrt_profile(str(output_dir).encode())
            print(f"profile: {n} file(s) written to {output_dir}", file=sys.stderr)

    mod.set_axon_ntff_profile_hook(_hook)


def run(inputs, trace=False):
    if trace:
        _install_profile_hook()
    in_maps, masked = _prep_in_maps(inputs)
    nc = _build(masked)
    res = bass_utils.run_bass_kernel_spmd(
        nc, in_maps, core_ids=list(range(NCORES)), trace=trace
    )
    out = np.zeros((B, S, D), np.float32)
    for c in range(NCORES):
        out[c // HG] += res.results[c]["out"]
    return out, res


def kernel(**inputs):
    return run(inputs, trace=False)[0]
